# revision 1
# baseline (speedup 1.0000x reference)
"""DOSLoss kernel for Trainium2, 8 NeuronCores, pure data parallel.

Loss = mean|out-scaled|
     + 0.05 * mean|scaling - scaling_factor|
     + 0.005 * mean|cumsum(out,1) - cumsum(scaled,1)|
     + 0.15 * mean|features - dos_features(x, out*scaling[:,None])|

Strategy per core (16384 rows x 400 cols, fp32):
 - stream row-tile pairs, one [128, 2, 400] DMA per tensor per pair
 - DVE: c = cumsum(out - scaled) fused into tensor_tensor_scan (the
   serial recurrence makes DVE the bottleneck engine, ~3 cyc/elem),
   plus half of the |cumsum| row-reduce
 - Pool (GpSimd): d = out - scaled via tensor_tensor
 - ACT: Abs+accumulate row sums of d and of the other half of c, the
   PSUM->SBUF abs-cast copy of the transposed out chunks, and the psum
   moment evacuation; all its cross-engine consumers are emitted one
   pair late so the in-order ACT stream never stalls on DVE/Pool
 - PE: transpose out-chunks (fp32), then matmul with [x^k, window]
   weights (fp16) -> 6 weighted row-sums that determine DOS features
 - end: bulk fp32 feature math on [128, nt] panels, l1 partials, one
   [128, 8] partial-sum tensor out per core; host combines 8 cores.
"""

from contextlib import ExitStack

import numpy as np

import concourse.bacc as bacc
import concourse.bass as bass
import concourse.mybir as mybir
import concourse.tile as tile
from concourse.bass_utils import run_bass_kernel_spmd

F32 = mybir.dt.float32
F16 = mybir.dt.float16
ALU = mybir.AluOpType
AF = mybir.ActivationFunctionType
AX = mybir.AxisListType

N_DOS = 400
N_CORES = 8
B_FULL = 131072
ROWS = B_FULL // N_CORES  # 16384 rows per core
DX = 20.0 / (N_DOS - 1)
ZERO_IDX = 199
SCALING_W = 0.05
CUMSUM_W = 0.005
FEATURES_W = 0.15

NCH = 4  # n-chunks per row tile
CW = 100  # chunk width (400 = 4*100)

# Column layout of the per-core partial output [128, 8]:
# 0: sum|out-scaled|, 1: sum|cumsum diff|, 2: sum|features-feats|,
# 3: sum|scaling-scaling_factor|; 4-7 unused.


def _u_np() -> np.ndarray:
    # U[r, c*400 + n'] = 1 if (100c + r) <= n'   (inclusive-cumsum matrix,
    # chunked by 100 contraction rows)
    u = np.zeros((128, NCH * N_DOS), np.float16)
    for c in range(NCH):
        for r in range(CW):
            n = c * CW + r
            u[r, c * N_DOS + n :(c + 1) * N_DOS] = 1.0
    return u


def _weights_np() -> np.ndarray:
    x = -10.0 + DX * np.arange(N_DOS, dtype=np.float64)
    w = np.zeros((N_DOS, 6), np.float64)
    for k in range(5):
        w[:, k] = x**k
    w[ZERO_IDX - 20 : ZERO_IDX + 20, 5] = 1.0
    wsb = np.zeros((128, NCH * 6), np.float16)
    for c in range(NCH):
        wsb[0:CW, c * 6 : (c + 1) * 6] = w[c * CW : (c + 1) * CW, :].astype(np.float16)
    return wsb


def build_nc(rows: int = ROWS, gpsimd_absacc: bool = True) -> bass.Bass:
    nt = rows // 128  # row tiles
    npair = nt // 2
    assert nt % 2 == 0

    nc = bacc.Bacc()
    d_out = nc.dram_tensor("x_out", [rows, N_DOS], F32, kind="ExternalInput")
    d_scaled = nc.dram_tensor("x_scaled", [rows, N_DOS], F32, kind="ExternalInput")
    d_scaling = nc.dram_tensor("x_scaling", [rows], F32, kind="ExternalInput")
    d_sf = nc.dram_tensor("x_sf", [rows], F32, kind="ExternalInput")
    d_feat = nc.dram_tensor("x_feat", [rows, 5], F32, kind="ExternalInput")
    d_w = nc.dram_tensor("w_const", [128, NCH * 6], F16, kind="ExternalInput")
    d_u = nc.dram_tensor("u_const", [128, NCH * N_DOS], F16, kind="ExternalInput")
    d_ident = nc.dram_tensor("ident", [128, 128], F32, kind="ExternalInput")
    d_i16 = nc.dram_tensor("ident16", [128, 128], F16, kind="ExternalInput")
    d_res = nc.dram_tensor("partials", [128, 8], F32, kind="ExternalOutput")

    with tile.TileContext(nc) as tc:
        with ExitStack() as ctx:
            const_pool = ctx.enter_context(tc.tile_pool(name="const", bufs=1))
            pers_pool = ctx.enter_context(tc.tile_pool(name="pers", bufs=1))
            io_pool = ctx.enter_context(tc.tile_pool(name="io", bufs=6))
            wk_pool = ctx.enter_context(tc.tile_pool(name="wk", bufs=6))
            at_pool = ctx.enter_context(tc.tile_pool(name="at", bufs=4))
            psT_pool = ctx.enter_context(
                tc.tile_pool(name="psT", bufs=2, space="PSUM")
            )
            ps6_pool = ctx.enter_context(
                tc.tile_pool(name="ps6", bufs=2, space="PSUM")
            )
            fin_pool = ctx.enter_context(tc.tile_pool(name="fin", bufs=1))

            ident = const_pool.tile([128, 128], F32, tag="ident")
            nc.sync.dma_start(ident[:], d_ident[:])
            wsb = const_pool.tile([128, NCH * 6], F16, tag="wsb")
            nc.sync.dma_start(wsb[:], d_w[:])
            i16 = const_pool.tile([128, 128], F16, tag="i16")
            nc.sync.dma_start(i16[:], d_i16[:])
            u_sb = const_pool.tile([128, NCH * N_DOS], F16, tag="u_sb")
            nc.sync.dma_start(u_sb[:], d_u[:])

            # Dummy PE ops so the PE sequencer observes the const-load DMA
            # lanes before the loop: the PE LW struct only supports a single
            # sync-wait, so steady-state transposes/matmuls must carry at
            # most one wait each.
            scr_ps = psT_pool.tile([128, 512], F32, tag="pT", name="scr_ps")
            nc.tensor.transpose(scr_ps[:, 0:128], ident[:], ident[:])
            scr_ps6 = ps6_pool.tile([128, 12], F32, tag="ps6", name="scr_ps6")
            nc.tensor.matmul(
                scr_ps6[0:24, 0:6], lhsT=wsb[0:100, 0:24], rhs=wsb[0:100, 0:6]
            )


            dsum = pers_pool.tile([128, npair], F32, tag="dsum")
            csum = pers_pool.tile([128, npair], F32, tag="csum")
            csum2 = pers_pool.tile([128, npair], F32, tag="csum2")
            s6 = pers_pool.tile([128, nt * 6], F32, tag="s6")

            src_o = d_out.rearrange("(pp q j) n -> q pp j n", j=2, q=128)
            src_s = d_scaled.rearrange("(pp q j) n -> q pp j n", j=2, q=128)

            # ACT ops consuming other engines' slow outputs are emitted one
            # pair late, so the in-order scalar-engine stream never stalls on
            # the scans (DVE) or the subtract (Pool).
            pending = []

            pT2_pool = ctx.enter_context(
                tc.tile_pool(name="pT2", bufs=2, space="PSUM")
            )
            cps_pool = ctx.enter_context(
                tc.tile_pool(name="cps", bufs=2, space="PSUM")
            )
            dt_pool = ctx.enter_context(tc.tile_pool(name="dt", bufs=3))

            def flush_pending():
                if not pending:
                    return
                pp, d2_o, ps6_o = pending.pop()
                scr_a = wk_pool.tile([128, 800], F32, tag="scrA", name="scrA")
                nc.scalar.activation(
                    scr_a[:], d2_o[:], AF.Abs, accum_out=dsum[:, pp : pp + 1]
                )
                nc.scalar.copy(s6[:, (2 * pp) * 6 : (2 * pp + 2) * 6], ps6_o[:])

            # PE-side cumsum of tile j=1: transposed-d (from pair p) is
            # evacuated and matmul'd against the triangular U one pair later,
            # and its |c| reduce runs one pair after that, so neither DVE nor
            # PE ever stalls on the chain Pool->PE->DVE->PE->DVE.
            pend_mm = []
            pend_red = []

            scr_p16 = pT2_pool.tile([128, 512], F16, tag="pT2", name="scr_p16")
            nc.tensor.transpose(scr_p16[:, 0:128], i16[:], i16[:])
            scr_cps = cps_pool.tile([128, 400], F32, tag="cps", name="scr_cps")
            nc.tensor.matmul(
                scr_cps[0:24, 0:6], lhsT=u_sb[0:100, 0:24], rhs=u_sb[0:100, 0:6]
            )

            def flush_cumsum_mm():
                if not pend_mm:
                    return
                pp, pT2_o = pend_mm.pop()
                d_t = dt_pool.tile([128, 512], F16, tag="dT", name="dT")
                nc.vector.tensor_copy(d_t[0:CW, :], pT2_o[0:CW, :])
                c_ps = cps_pool.tile([128, 400], F32, tag="cps", name="cps")
                for c in range(NCH):
                    nc.tensor.matmul(
                        c_ps[:],
                        lhsT=d_t[0:CW, c * 128 : (c + 1) * 128],
                        rhs=u_sb[0:CW, c * N_DOS : (c + 1) * N_DOS],
                        start=(c == 0),
                        stop=(c == NCH - 1),
                    )
                pend_red.append((pp, c_ps))

            def flush_cumsum_red():
                if not pend_red:
                    return
                pp, c_ps = pend_red.pop(0)
                nc.vector.tensor_reduce(
                    csum2[:, pp : pp + 1],
                    c_ps[:],
                    axis=AX.X,
                    op=ALU.add,
                    apply_absolute_value=True,
                )

            for p in range(npair):
                o2 = io_pool.tile([128, 800], F32, tag="o2")
                s2 = io_pool.tile([128, 800], F32, tag="s2")
                nc.sync.dma_start(
                    o2[:].rearrange("q (j n) -> q j n", j=2), src_o[:, p]
                )
                nc.scalar.dma_start(
                    s2[:].rearrange("q (j n) -> q j n", j=2), src_s[:, p]
                )

                # cumsum of (out - scaled) for tile j=0, fused into the scan
                c2 = wk_pool.tile([128, 400], F32, tag="c2")
                nc.vector.tensor_tensor_scan(
                    c2[:],
                    o2[:, 0:400],
                    s2[:, 0:400],
                    0.0,
                    op0=ALU.add,
                    op1=ALU.subtract,
                )
                # d materialized on the Pool engine in fp16 (feeds both the
                # |d| accumulation and the PE-side cumsum of tile j=1)
                d2 = wk_pool.tile([128, 800], F16, tag="d2")
                if gpsimd_absacc:
                    nc.gpsimd.tensor_tensor(d2[:], o2[:], s2[:], op=ALU.subtract)
                else:
                    nc.vector.tensor_tensor(d2[:], o2[:], s2[:], op=ALU.subtract)
                ps6 = ps6_pool.tile([128, 12], F32, tag="ps6")
                for j in (0, 1):
                    p_t = psT_pool.tile([128, 512], F32, tag="pT")
                    for c in range(NCH):
                        nc.tensor.transpose(
                            p_t[0:CW, c * 128 : (c + 1) * 128],
                            o2[:, j * 400 + c * CW : j * 400 + (c + 1) * CW],
                            ident[:],
                        )
                    a_t = at_pool.tile([128, 512], F16, tag="aT")
                    nc.scalar.activation(a_t[0:CW, :], p_t[0:CW, :], AF.Abs)
                    for c in range(NCH):
                        nc.tensor.matmul(
                            ps6[:, j * 6 : (j + 1) * 6],
                            lhsT=a_t[0:CW, c * 128 : (c + 1) * 128],
                            rhs=wsb[0:CW, c * 6 : (c + 1) * 6],
                            start=(c == 0),
                            stop=(c == NCH - 1),
                        )
                # transpose d (tile j=1 columns) for the PE-side cumsum
                pT2 = pT2_pool.tile([128, 512], F16, tag="pT2", name="pT2")
                for c in range(NCH):
                    nc.tensor.transpose(
                        pT2[0:CW, c * 128 : (c + 1) * 128],
                        d2[:, 400 + c * CW : 400 + (c + 1) * CW],
                        i16[:],
                    )
                nc.vector.tensor_reduce(
                    csum[:, p : p + 1],
                    c2[:],
                    axis=AX.X,
                    op=ALU.add,
                    apply_absolute_value=True,
                )
                flush_cumsum_mm()
                flush_cumsum_red()
                flush_pending()
                pending.append((p, d2, ps6))
                pend_mm.append((p, pT2))
            flush_cumsum_mm()
            flush_cumsum_red()
            flush_cumsum_red()
            flush_pending()

            # ---- end phase: features math ----
            sc_t = fin_pool.tile([128, nt], F32, tag="sc_t")
            nc.sync.dma_start(
                sc_t[:].rearrange("q (pp j) -> q pp j", j=2),
                d_scaling.rearrange("(pp q j) -> q pp j", q=128, j=2),
            )
            sc_c = fin_pool.tile([128, nt], F32, tag="sc_c")
            nc.sync.dma_start(sc_c[:], d_scaling.rearrange("(q t) -> q t", t=nt))
            sf_c = fin_pool.tile([128, nt], F32, tag="sf_c")
            nc.sync.dma_start(sf_c[:], d_sf.rearrange("(q t) -> q t", t=nt))
            fv = fin_pool.tile([128, nt * 5], F32, tag="fv")
            nc.sync.dma_start(
                fv[:].rearrange("q (pp j f) -> q pp j f", j=2, f=5),
                d_feat.rearrange("(pp q j) f -> q pp j f", q=128, j=2),
            )

            s6v = s6[:].rearrange("q (t k) -> q k t", k=6)

            def ftile(tag):
                return fin_pool.tile([128, nt], F32, tag=tag, name=tag)

            r0 = ftile("r0")
            nc.vector.reciprocal(r0[:], s6v[:, 0])
            cc = ftile("cc")
            nc.vector.tensor_tensor(cc[:], s6v[:, 1], r0[:], op=ALU.mult)
            r2 = ftile("r2")
            nc.vector.tensor_tensor(r2[:], s6v[:, 2], r0[:], op=ALU.mult)
            r3 = ftile("r3")
            nc.vector.tensor_tensor(r3[:], s6v[:, 3], r0[:], op=ALU.mult)
            r4 = ftile("r4")
            nc.vector.tensor_tensor(r4[:], s6v[:, 4], r0[:], op=ALU.mult)
            csq = ftile("csq")
            nc.vector.tensor_tensor(csq[:], cc[:], cc[:], op=ALU.mult)
            wid = ftile("wid")
            nc.vector.tensor_tensor(wid[:], r2[:], csq[:], op=ALU.subtract)
            rw = ftile("rw")
            nc.vector.reciprocal(rw[:], wid[:])
            sq = ftile("sq")
            nc.scalar.activation(sq[:], rw[:], AF.Sqrt)  # sqrt(1/w)
            rw15 = ftile("rw15")
            nc.vector.tensor_tensor(rw15[:], rw[:], sq[:], op=ALU.mult)  # w^-1.5
            rw2 = ftile("rw2")
            nc.vector.tensor_tensor(rw2[:], rw[:], rw[:], op=ALU.mult)  # w^-2

            # skew = (r3 - 3 c r2 + 2 c^3) * w^-1.5
            t3 = ftile("t3")
            nc.vector.scalar_tensor_tensor(
                t3[:], cc[:], 3.0, r2[:], op0=ALU.mult, op1=ALU.mult
            )  # 3 c r2
            t4 = ftile("t4")
            nc.vector.tensor_tensor(t4[:], r3[:], t3[:], op=ALU.subtract)
            c3 = ftile("c3")
            nc.vector.tensor_tensor(c3[:], csq[:], cc[:], op=ALU.mult)
            skn = ftile("skn")
            nc.vector.scalar_tensor_tensor(
                skn[:], c3[:], 2.0, t4[:], op0=ALU.mult, op1=ALU.add
            )
            skew = ftile("skew")
            nc.vector.tensor_tensor(skew[:], skn[:], rw15[:], op=ALU.mult)

            # kurt = (r4 - 4 c r3 + 6 c^2 r2 - 3 c^4) * w^-2
            u1 = ftile("u1")
            nc.vector.scalar_tensor_tensor(
                u1[:], cc[:], 4.0, r3[:], op0=ALU.mult, op1=ALU.mult
            )
            u2 = ftile("u2")
            nc.vector.tensor_tensor(u2[:], r4[:], u1[:], op=ALU.subtract)
            u3 = ftile("u3")
            nc.vector.scalar_tensor_tensor(
                u3[:], csq[:], 6.0, r2[:], op0=ALU.mult, op1=ALU.mult
            )
            u4 = ftile("u4")
            nc.vector.tensor_tensor(u4[:], u2[:], u3[:], op=ALU.add)
            u5 = ftile("u5")
            nc.vector.scalar_tensor_tensor(
                u5[:], csq[:], 3.0, csq[:], op0=ALU.mult, op1=ALU.mult
            )
            kn = ftile("kn")
            nc.vector.tensor_tensor(kn[:], u4[:], u5[:], op=ALU.subtract)
            kurt = ftile("kurt")
            nc.vector.tensor_tensor(kurt[:], kn[:], rw2[:], op=ALU.mult)

            ef = ftile("ef")
            nc.vector.scalar_tensor_tensor(
                ef[:], s6v[:, 5], DX, sc_t[:], op0=ALU.mult, op1=ALU.mult
            )

            # |features - feats| accumulated
            fdiff = fin_pool.tile([128, nt * 5], F32, tag="fdiff")
            fdv = fdiff[:].rearrange("q (t f) -> q f t", f=5)
            fvv = fv[:].rearrange("q (t f) -> q f t", f=5)
            feats = [cc, wid, skew, kurt, ef]
            for k in range(5):
                nc.vector.tensor_tensor(
                    fdv[:, k], fvv[:, k], feats[k][:], op=ALU.subtract
                )
            fs = fin_pool.tile([128, 8], F32, tag="fs")
            scr_f = fin_pool.tile([128, nt * 5], F32, tag="scrF")
            nc.scalar.activation(
                scr_f[:], fdiff[:], AF.Abs, accum_out=fs[:, 2:3]
            )

            dsc = fin_pool.tile([128, nt], F32, tag="dsc")
            nc.vector.tensor_tensor(dsc[:], sc_c[:], sf_c[:], op=ALU.subtract)
            scr_s = fin_pool.tile([128, nt], F32, tag="scrS")
            nc.scalar.activation(
                scr_s[:], dsc[:], AF.Abs, accum_out=fs[:, 3:4]
            )

            nc.vector.tensor_reduce(fs[:, 0:1], dsum[:], axis=AX.X, op=ALU.add)
            nc.vector.tensor_reduce(fs[:, 1:2], csum[:], axis=AX.X, op=ALU.add)
            nc.vector.tensor_reduce(fs[:, 4:5], csum2[:], axis=AX.X, op=ALU.add)
            nc.gpsimd.memset(fs[:, 5:8], 0.0)

            nc.sync.dma_start(d_res[:], fs[:])

    nc.compile()
    return nc


_NC_CACHE: dict = {}


def _get_nc(rows: int, gpsimd_absacc: bool = True) -> bass.Bass:
    key = (rows, gpsimd_absacc)
    if key not in _NC_CACHE:
        _NC_CACHE[key] = build_nc(rows, gpsimd_absacc)
    return _NC_CACHE[key]


def make_in_maps(out, scaling, scaled, scaling_factor, features, n_cores=N_CORES):
    rows = out.shape[0] // n_cores
    wsb = _weights_np()
    u = _u_np()
    ident = np.eye(128, dtype=np.float32)
    i16 = np.eye(128, dtype=np.float16)
    in_maps = []
    for i in range(n_cores):
        sl = slice(i * rows, (i + 1) * rows)
        in_maps.append(
            {
                "x_out": np.ascontiguousarray(out[sl]),
                "x_scaled": np.ascontiguousarray(scaled[sl]),
                "x_scaling": np.ascontiguousarray(scaling[sl]),
                "x_sf": np.ascontiguousarray(scaling_factor[sl]),
                "x_feat": np.ascontiguousarray(features[sl]),
                "w_const": wsb,
                "u_const": u,
                "ident": ident,
                "ident16": i16,
            }
        )
    return in_maps


def combine_partials(partials_list, b_full: int) -> np.float32:
    tot = np.zeros(5, np.float64)
    for fs in partials_list:
        tot += fs[:, 0:5].astype(np.float64).sum(axis=0)
    tot[1] += tot[4]
    dos_loss = tot[0] / (b_full * N_DOS)
    cumsum_loss = tot[1] / (b_full * N_DOS)
    features_loss = tot[2] / (b_full * 5)
    scaling_loss = tot[3] / b_full
    return np.float32(
        dos_loss
        + SCALING_W * scaling_loss
        + CUMSUM_W * cumsum_loss
        + FEATURES_W * features_loss
    )


def kernel(out, scaling, scaled, scaling_factor, features):
    out = np.asarray(out, np.float32)
    scaling = np.asarray(scaling, np.float32)
    scaled = np.asarray(scaled, np.float32)
    scaling_factor = np.asarray(scaling_factor, np.float32)
    features = np.asarray(features, np.float32)

    nc = _get_nc(ROWS)
    in_maps = make_in_maps(out, scaling, scaled, scaling_factor, features)
    res = run_bass_kernel_spmd(nc, in_maps, list(range(N_CORES)))
    partials = [res.results[i]["partials"] for i in range(N_CORES)]
    return combine_partials(partials, out.shape[0])


if __name__ == "__main__":
    rng = np.random.default_rng(0)
    B = 2048
    o = rng.standard_normal((B, N_DOS), dtype=np.float32)
    sc = (rng.random(B, dtype=np.float32) + 0.5).astype(np.float32)
    sd = rng.standard_normal((B, N_DOS), dtype=np.float32)
    sf = (rng.random(B, dtype=np.float32) + 0.5).astype(np.float32)
    ft = rng.standard_normal((B, 5), dtype=np.float32)
    print("building...")
    nc = build_nc(B)
    print("instructions built ok")

